# revision 1
# baseline (speedup 1.0000x reference)
"""Trainium2 Bass kernel for nn_CFGSubASTExpressionCombiner.

Segment-softmax attention pooling over ragged groups:
  attn_keys = scatter(ast[pdg_val]) by pdg_key (last-write-wins)
  x = ast[map_key]  [M, D]
  per CFG node c: softmax-weighted pooling of v = x@Wv rows whose seg == c,
  with per-head scores k.q (q from attn_keys), then @ Wo + bo.

Strategy (v3): host sorts mapping entries by segment id, assigns each of the
8 cores a contiguous range of segments (no collectives), and PRE-GATHERS
everything the device needs into dense per-pair records (pure data movement /
layout work on host; all matmul/softmax flops stay on device):
  - xT tiles [128(dlow), (dhalf, m)] quantized two-level fp8-e4m3:
    xhi = fp8(x), xlo = fp8(x - xhi).  k = xhi@(64*Wk) via one DoubleRow
    matmul (2x rate).  v*64 = xhi@Wvhi + xlo@Wvhi + xhi@Wvlo (three DR
    matmuls; Wvhi = fp8(64*Wv), Wvlo = fp8(64*Wv - Wvhi)); the 64x scales
    cancel via the exp() scale (k) and a 1/64-scaled Wo (v).  End-to-end
    rel err ~5e-3, same as all-bf16.
  - one-hot A [m,c] / AT [c,m] selection matrices per tile (bf16)
  - masked, transposed attention-key encodings per block
Per block (<=128 segments, <=2048 entries = 16 tiles = 8 pairs):
  q = keysT^T @ Wq (PE); per pair of tiles (software-pipelined emission):
    k (fp8 DR mm), v (fp8 DR mms), qg = AT^T @ q (PE gather-by-matmul)
    k,qg -> SBUF bf16 (Act engine copies), scores = rowsum4(k*qg) (DVE
    2x mult + reduce), e = exp(scores/512) (Act, lag 1 pair),
    rhs = [e*v | e] (DVE), nd += A^T @ rhs (PE, single PSUM accumulation
    group per bank, emitted with a 3-pair lag to cover the DVE/Act chain)
  pooled = nd_num/(nd_den+1e-9); out = pooledT^T @ (Wo/64) (PE).
No segment-max subtraction (scores bounded for this problem's scale).
"""
import sys

sys.path.insert(0, "/opt/trn_rl_repo")

from contextlib import ExitStack

import ml_dtypes
import numpy as np

import concourse.bass as bass
import concourse.tile as tile
from concourse import bacc, mybir
from concourse.bass_utils import run_bass_kernel_spmd
from concourse.masks import make_identity

P = 128
D = 256
H = 4
DH = 64
OUT_D = 256
NCORES = 8
TPB = 16          # tiles per block (8 pairs)
EPB = TPB * P     # entries per block capacity
KSCALE = 64.0     # Wk/Wv prescale for fp8 range
bf16 = mybir.dt.bfloat16
f32 = mybir.dt.float32
f8 = mybir.dt.float8e4
e4m3 = ml_dtypes.float8_e4m3

_nc_cache = {}


def _host_prep(map_key, seg, C):
    """Sort entries by segment, split segments across cores, pack blocks."""
    M = seg.shape[0]
    order = np.argsort(seg, kind="stable")
    seg_s = seg[order].astype(np.int64)
    gid_s = map_key[order].astype(np.int64)
    counts = np.bincount(seg_s, minlength=C)
    cum = np.concatenate([[0], np.cumsum(counts)])  # cum[c] = entries with seg < c

    bounds = [0]
    for r in range(1, NCORES):
        c = int(np.searchsorted(cum, M * r / NCORES))
        bounds.append(max(bounds[-1], min(c, C)))
    bounds.append(C)

    cores = []  # per core: list of (seg_base, nseg)
    for r in range(NCORES):
        c0, c1 = bounds[r], bounds[r + 1]
        blocks = []
        c = c0
        while c < c1:
            nseg, nent = 0, 0
            while c + nseg < c1 and nseg < P:
                cnt = int(counts[c + nseg])
                if nent + cnt > EPB and nseg > 0:
                    break
                assert cnt <= EPB
                nent += cnt
                nseg += 1
            blocks.append((c, nseg))
            c += nseg
        cores.append(blocks)
    nblk = max(len(b) for b in cores)

    njt = nblk * TPB
    gidx = np.zeros((NCORES, njt, P), np.int64)    # source ast row per slot
    segl = np.full((NCORES, njt, P), -1, np.int64)  # block-local segment id
    for r in range(NCORES):
        for b, (base, nseg) in enumerate(cores[r]):
            s, e = cum[base], cum[base + nseg]
            n = e - s
            g = np.zeros(EPB, np.int64)
            sl = np.full(EPB, -1, np.int64)
            g[:n] = gid_s[s:e]
            sl[:n] = seg_s[s:e] - base
            gidx[r, b * TPB:(b + 1) * TPB] = g.reshape(TPB, P)
            segl[r, b * TPB:(b + 1) * TPB] = sl.reshape(TPB, P)
    return cores, cum, nblk, gidx, segl


def _build(nblk, has_bq, has_bkv, has_bo):
    key = (nblk, has_bq, has_bkv, has_bo)
    if key in _nc_cache:
        return _nc_cache[key]
    npr = nblk * (TPB // 2)
    nc = bacc.Bacc("TRN2", target_bir_lowering=False, debug=False,
                   num_devices=NCORES)

    # per-pair record, bf16 cols: [A0|A1|AT0|AT1 (512)] then fp8 bytes viewed
    # as bf16: [xhi0|xhi1 (256)] [xlo0|xlo1 (256)]
    recp_d = nc.dram_tensor("recp", [npr, P, 1024], bf16, kind="ExternalInput").ap()
    keysT_d = nc.dram_tensor("keysT", [nblk, P, D], bf16, kind="ExternalInput").ap()
    wk8_d = nc.dram_tensor("wk8", [P, 2 * D], f8, kind="ExternalInput").ap()
    wvhi_d = nc.dram_tensor("wvhi", [P, 2 * D], f8, kind="ExternalInput").ap()
    wvlo_d = nc.dram_tensor("wvlo", [P, 2 * D], f8, kind="ExternalInput").ap()
    wq_d = nc.dram_tensor("wq", [2, P, D], bf16, kind="ExternalInput").ap()
    wo_d = nc.dram_tensor("wo", [2, P, OUT_D], bf16, kind="ExternalInput").ap()
    bq_d = nc.dram_tensor("bq", [1, D], bf16, kind="ExternalInput").ap()
    bkv_d = nc.dram_tensor("bkv", [1, 2 * D], bf16, kind="ExternalInput").ap()
    bo_d = nc.dram_tensor("bo", [1, OUT_D], bf16, kind="ExternalInput").ap()
    out_d = nc.dram_tensor("out", [nblk * P, OUT_D], f32, kind="ExternalOutput").ap()

    def f8view(ap_slice):
        return ap_slice.bitcast(f8).rearrange("p (t m) -> p t m", t=2)

    with tile.TileContext(nc) as tc:
        with ExitStack() as ctx:
            cp = ctx.enter_context(tc.tile_pool(name="const", bufs=1))
            tp = ctx.enter_context(tc.tile_pool(name="tp", bufs=8))
            bp = ctx.enter_context(tc.tile_pool(name="bp", bufs=4))
            # PSUM budget is 8 banks x 2KB: k 2 + v 3 + qg 1 + nd 2 = 8.
            # k frees early (scalar copy), v late (e*v) -> v gets more bufs.
            kp = ctx.enter_context(tc.tile_pool(name="kp", bufs=2, space="PSUM"))
            vp = ctx.enter_context(tc.tile_pool(name="vp", bufs=3, space="PSUM"))
            qgp = ctx.enter_context(tc.tile_pool(name="qgp", bufs=1, space="PSUM"))
            ndp = ctx.enter_context(tc.tile_pool(name="ndp", bufs=2, space="PSUM"))

            ident = cp.tile([P, P], bf16)
            make_identity(nc, ident[:])
            wk8_sb = cp.tile([P, 2 * D], f8)
            nc.sync.dma_start(out=wk8_sb[:], in_=wk8_d[:, :])
            wvhi_sb = cp.tile([P, 2 * D], f8)
            nc.sync.dma_start(out=wvhi_sb[:], in_=wvhi_d[:, :])
            wvlo_sb = cp.tile([P, 2 * D], f8)
            nc.sync.dma_start(out=wvlo_sb[:], in_=wvlo_d[:, :])
            wq0 = cp.tile([P, D], bf16)
            wq1 = cp.tile([P, D], bf16)
            nc.sync.dma_start(out=wq0[:], in_=wq_d[0])
            nc.sync.dma_start(out=wq1[:], in_=wq_d[1])
            wo0 = cp.tile([P, OUT_D], bf16)
            wo1 = cp.tile([P, OUT_D], bf16)
            nc.sync.dma_start(out=wo0[:], in_=wo_d[0])
            nc.sync.dma_start(out=wo1[:], in_=wo_d[1])
            if has_bq or has_bkv or has_bo:
                ones1 = cp.tile([1, P], bf16)
                nc.gpsimd.memset(ones1[:], 1.0)
            if has_bq:
                bq_r = cp.tile([1, D], bf16)
                nc.sync.dma_start(out=bq_r[:], in_=bq_d[:, :])
            if has_bkv:
                bkv_r = cp.tile([1, 2 * D], bf16)
                nc.sync.dma_start(out=bkv_r[:], in_=bkv_d[:, :])
            if has_bo:
                bo_r = cp.tile([1, OUT_D], bf16)
                nc.sync.dma_start(out=bo_r[:], in_=bo_d[:, :])

            wk8_v = wk8_sb[:, :].rearrange("p (t n) -> p t n", t=2)
            wvhi_v = wvhi_sb[:, :].rearrange("p (t n) -> p t n", t=2)
            wvlo_v = wvlo_sb[:, :].rearrange("p (t n) -> p t n", t=2)

            # ---- q computation for one block (emitted 2 blocks ahead) ----
            q_tiles = {}

            def emit_q(b):
                kT = bp.tile([P, D], bf16, tag="kT")
                nc.sync.dma_start(out=kT[:], in_=keysT_d[b])
                q_ps = qgp.tile([P, D], f32, tag="qg")
                nc.tensor.matmul(out=q_ps[:], lhsT=kT[:, 0:P], rhs=wq0[:],
                                 start=True, stop=False)
                nc.tensor.matmul(out=q_ps[:], lhsT=kT[:, P:D], rhs=wq1[:],
                                 start=False, stop=not has_bq)
                if has_bq:
                    nc.tensor.matmul(out=q_ps[:], lhsT=ones1[:], rhs=bq_r[:],
                                     start=False, stop=True)
                q_sb = cp.tile([P, D], bf16, tag=f"q{b}")
                nc.scalar.copy(out=q_sb[:], in_=q_ps[:])
                q_tiles[b] = q_sb

            emit_q(0)
            if nblk > 1:
                emit_q(1)

            nd_tiles = {}
            pend_ev = []   # pairs awaiting exp + e*v emission (lag 1)
            pend_nd = []   # pairs awaiting nd emission (lag 3)

            def emit_ev(st):
                rhs_t, scores, v_ps = st["rhs"], st["sc"], st["v"]
                nc.scalar.activation(
                    out=rhs_t[:, :].rearrange("p (t c) -> p t c",
                                              t=2)[:, :, D:D + H],
                    in_=scores[:, :].rearrange("p (t h) -> p t h", t=2),
                    func=mybir.ActivationFunctionType.Exp,
                    scale=float(1.0 / (KSCALE * np.sqrt(DH))))
                rview = rhs_t[:, :].rearrange("p (t c) -> p t c", t=2)
                nc.vector.tensor_tensor(
                    out=rview[:, :, 0:D].rearrange("p t (h x) -> p t h x", x=DH),
                    in0=v_ps[:, :].rearrange("p (t h x) -> p t h x", t=2, h=H),
                    in1=rview[:, :, D:D + H][:, :, :, None].to_broadcast(
                        [P, 2, H, DH]),
                    op=mybir.AluOpType.mult)

            def block_end(b):
                nd_ps = nd_tiles.pop(b)
                dsb = bp.tile([P, H], f32, tag="dsb")
                nc.vector.tensor_scalar(out=dsb[:], in0=nd_ps[:, D:D + H],
                                        scalar1=1e-9, scalar2=None,
                                        op0=mybir.AluOpType.add)
                recip = bp.tile([P, H], f32, tag="recip")
                nc.vector.reciprocal(out=recip[:], in_=dsb[:])
                pooled = bp.tile([P, D], bf16, tag="pooled")
                nc.vector.tensor_tensor(
                    out=pooled[:, :].rearrange("p (h x) -> p h x", x=DH),
                    in0=nd_ps[:, 0:D].rearrange("p (h x) -> p h x", x=DH),
                    in1=recip[:, :, None].to_broadcast([P, H, DH]),
                    op=mybir.AluOpType.mult)
                pT_ps = qgp.tile([P, D], bf16, tag="qg")
                nc.tensor.transpose(out=pT_ps[:, 0:P], in_=pooled[:, 0:P],
                                    identity=ident[:])
                nc.tensor.transpose(out=pT_ps[:, P:D], in_=pooled[:, P:D],
                                    identity=ident[:])
                pooledT = bp.tile([P, D], bf16, tag="pT")
                nc.scalar.copy(out=pooledT[:], in_=pT_ps[:])
                o_ps = kp.tile([P, OUT_D], f32, tag="k")
                nc.tensor.matmul(out=o_ps[:], lhsT=pooledT[:, 0:P], rhs=wo0[:],
                                 start=True, stop=False)
                nc.tensor.matmul(out=o_ps[:], lhsT=pooledT[:, P:D], rhs=wo1[:],
                                 start=False, stop=not has_bo)
                if has_bo:
                    nc.tensor.matmul(out=o_ps[:], lhsT=ones1[:], rhs=bo_r[:],
                                     start=False, stop=True)
                out_sb = bp.tile([P, OUT_D], f32, tag="out")
                nc.scalar.copy(out=out_sb[:], in_=o_ps[:])
                nc.sync.dma_start(out=out_d[b * P:(b + 1) * P, :], in_=out_sb[:])

            def emit_nd2(st):
                # single PSUM accumulation group [P, 260]: two open groups in
                # one bank corrupt the accumulation on HW
                b, pr = st["b"], st["pr"]
                nd_ps = nd_tiles[b]
                rec_t, rhs_t = st["rec"], st["rhs"]
                first, last = pr == 0, pr == TPB // 2 - 1
                nc.tensor.matmul(out=nd_ps[:, 0:D + H], lhsT=rec_t[:, 0:P],
                                 rhs=rhs_t[:, 0:D + H], start=first, stop=False)
                nc.tensor.matmul(out=nd_ps[:, 0:D + H], lhsT=rec_t[:, P:2 * P],
                                 rhs=rhs_t[:, D + H:2 * (D + H)],
                                 start=False, stop=last)
                if last:
                    block_end(b)

            for b in range(nblk):
                if b + 2 < nblk:
                    emit_q(b + 2)
                q_sb = q_tiles.pop(b)[:, :]
                nd_tiles[b] = ndp.tile([P, D + H], f32, tag="nd", name=f"nd{b}")

                for pr in range(TPB // 2):
                    t = b * (TPB // 2) + pr
                    rec_t = tp.tile([P, 1024], bf16, tag="rec")
                    nc.sync.dma_start(out=rec_t[:, 0:512],
                                      in_=recp_d[t, :, 0:512])
                    nc.gpsimd.dma_start(out=rec_t[:, 512:1024],
                                        in_=recp_d[t, :, 512:1024])

                    qg_ps = qgp.tile([P, 2 * D], f32, tag="qg")
                    nc.tensor.matmul(out=qg_ps[:, 0:D], lhsT=rec_t[:, 256:384],
                                     rhs=q_sb, start=True, stop=True)
                    nc.tensor.matmul(out=qg_ps[:, D:2 * D],
                                     lhsT=rec_t[:, 384:512],
                                     rhs=q_sb, start=True, stop=True)
                    k_ps = kp.tile([P, 512], f32, tag="k")
                    for j in range(2):
                        xhi_j = f8view(rec_t[:, 512 + j * P:512 + (j + 1) * P])
                        nc.tensor.matmul(
                            out=k_ps[:, j * D:(j + 1) * D], lhsT=xhi_j,
                            rhs=wk8_v,
                            perf_mode=mybir.MatmulPerfMode.DoubleRow,
                            start=True, stop=not has_bkv)
                        if has_bkv:
                            nc.tensor.matmul(out=k_ps[:, j * D:(j + 1) * D],
                                             lhsT=ones1[:], rhs=bkv_r[:, 0:D],
                                             start=False, stop=True)
                    qg_sb = tp.tile([P, 2 * D], bf16, tag="qgs")
                    nc.scalar.copy(out=qg_sb[:], in_=qg_ps[:])
                    k_sb = tp.tile([P, 2 * D], bf16, tag="ksb")
                    nc.scalar.copy(out=k_sb[:], in_=k_ps[:])
                    v_ps = vp.tile([P, 512], f32, tag="v")
                    for j in range(2):
                        o = j * D
                        xhi_j = f8view(rec_t[:, 512 + j * P:512 + (j + 1) * P])
                        xlo_j = f8view(rec_t[:, 768 + j * P:768 + (j + 1) * P])
                        nc.tensor.matmul(out=v_ps[:, o:o + D], lhsT=xhi_j,
                                         rhs=wvhi_v,
                                         perf_mode=mybir.MatmulPerfMode.DoubleRow,
                                         start=True, stop=False)
                        nc.tensor.matmul(out=v_ps[:, o:o + D], lhsT=xlo_j,
                                         rhs=wvhi_v,
                                         perf_mode=mybir.MatmulPerfMode.DoubleRow,
                                         start=False, stop=False)
                        nc.tensor.matmul(out=v_ps[:, o:o + D], lhsT=xhi_j,
                                         rhs=wvlo_v,
                                         perf_mode=mybir.MatmulPerfMode.DoubleRow,
                                         start=False, stop=not has_bkv)
                        if has_bkv:
                            nc.tensor.matmul(out=v_ps[:, o:o + D], lhsT=ones1[:],
                                             rhs=bkv_r[:, D:2 * D],
                                             start=False, stop=True)
                    tmp = tp.tile([P, 2 * D], bf16, tag="tmp")
                    nc.vector.tensor_tensor(out=tmp[:], in0=k_sb[:],
                                            in1=qg_sb[:], op=mybir.AluOpType.mult)
                    scores = tp.tile([P, 2 * H], f32, tag="sc")
                    nc.vector.reduce_sum(
                        out=scores[:, :].rearrange("p (t h) -> p t h", t=2),
                        in_=tmp[:, :].rearrange("p (t h x) -> p t h x", t=2, h=H),
                        axis=mybir.AxisListType.X)
                    rhs_t = tp.tile([P, 2 * (D + H)], bf16, tag="rhs")

                    pend_ev.append({"b": b, "pr": pr, "rec": rec_t, "rhs": rhs_t,
                                    "sc": scores, "v": v_ps})
                    if len(pend_ev) > 1:
                        st = pend_ev.pop(0)
                        emit_ev(st)
                        pend_nd.append(st)
                    if len(pend_nd) > 2:
                        emit_nd2(pend_nd.pop(0))
            for st in pend_ev:
                emit_ev(st)
                pend_nd.append(st)
            for st in pend_nd:
                emit_nd2(st)

    nc.compile()
    _nc_cache[key] = nc
    return nc


def kernel(**inputs):
    ast = np.ascontiguousarray(np.asarray(inputs["ast_nodes_encodings"], np.float32))
    map_key = np.asarray(inputs["ast_node_idx_to_pdg_node_idx_mapping_key"]).astype(np.int64)
    seg = np.asarray(inputs["ast_node_idx_to_pdg_node_idx_mapping_value"]).astype(np.int64)
    pdg_key = np.asarray(inputs["pdg_node_idx_to_sub_ast_root_idx_mapping_key"]).astype(np.int64)
    pdg_val = np.asarray(inputs["pdg_node_idx_to_sub_ast_root_idx_mapping_value"]).astype(np.int64)
    C = int(np.asarray(inputs["nr_cfg_nodes"]))
    Wq = np.asarray(inputs["Wq"], np.float32)
    bq = np.asarray(inputs["bq"], np.float32)
    Wk = np.asarray(inputs["Wk"], np.float32)
    bk = np.asarray(inputs["bk"], np.float32)
    Wv = np.asarray(inputs["Wv"], np.float32)
    bv = np.asarray(inputs["bv"], np.float32)
    Wo = np.asarray(inputs["Wo"], np.float32)
    bo = np.asarray(inputs["bo"], np.float32)

    cores, cum, nblk, gidx, segl = _host_prep(map_key, seg, C)
    njt = nblk * TPB
    npr = nblk * (TPB // 2)

    to_bf = lambda a: np.ascontiguousarray(a).astype(ml_dtypes.bfloat16)

    # xT tiles: [core, tile, dlow, (dhalf, m)], two-level fp8
    xs = ast[gidx]                                         # [8, njt, m, d] f32
    xhi = xs.astype(e4m3)
    xlo = (xs - xhi.astype(np.float32)).astype(e4m3)
    del xs

    def to_xt(a):                                          # [8, njt, m, d] -> xT
        return np.ascontiguousarray(
            a.reshape(NCORES, njt, P, 2, P).transpose(0, 1, 4, 3, 2)
        ).reshape(NCORES, njt, P, D)

    xt_hi = to_xt(xhi)
    xt_lo = to_xt(xlo)
    iota = np.arange(P)
    A = (segl[..., None] == iota)                          # [8, njt, m, c] bool
    AT = A.swapaxes(-1, -2)

    recp = np.zeros((NCORES, npr, P, 1024), ml_dtypes.bfloat16)
    recp[..., 0:128] = A[:, 0::2]
    recp[..., 128:256] = A[:, 1::2]
    recp[..., 256:384] = AT[:, 0::2]
    recp[..., 384:512] = AT[:, 1::2]
    ru16 = recp.view(np.uint16)
    ru16[..., 512:640] = xt_hi[:, 0::2].view(np.uint8).view(np.uint16)
    ru16[..., 640:768] = xt_hi[:, 1::2].view(np.uint8).view(np.uint16)
    ru16[..., 768:896] = xt_lo[:, 0::2].view(np.uint8).view(np.uint16)
    ru16[..., 896:1024] = xt_lo[:, 1::2].view(np.uint8).view(np.uint16)

    # attn_keys source resolution: last-write-wins scatter -> gather + mask
    src = np.zeros(C, np.int64)
    src[pdg_key] = pdg_val
    written = np.zeros(C, bool)
    written[pdg_key] = True
    ksrc = np.zeros((NCORES, nblk, P), np.int64)
    kmask = np.zeros((NCORES, nblk, P), bool)
    for r in range(NCORES):
        for b, (base, nseg) in enumerate(cores[r]):
            ksrc[r, b, :nseg] = src[base:base + nseg]
            kmask[r, b, :nseg] = written[base:base + nseg]
    keys = ast.astype(ml_dtypes.bfloat16)[ksrc]            # [8, nblk, c, d]
    keys[~kmask] = 0
    keysT = np.ascontiguousarray(
        keys.reshape(NCORES, nblk, P, 2, P).transpose(0, 1, 4, 3, 2)
    ).reshape(NCORES, nblk, P, D)

    def wtile(w):                                          # [256,256]->[128,512]
        return np.ascontiguousarray(
            w.reshape(2, P, D).transpose(1, 0, 2)).reshape(P, 2 * D)

    wk8 = wtile(Wk * KSCALE).astype(e4m3)
    wv64 = wtile(Wv * KSCALE)
    wvhi = wv64.astype(e4m3)
    wvlo = (wv64 - wvhi.astype(np.float32)).astype(e4m3)
    wq_b = np.stack([to_bf(Wq[0:P]), to_bf(Wq[P:2 * P])])
    wo_s = Wo / KSCALE
    wo_b = np.stack([to_bf(wo_s[0:P]), to_bf(wo_s[P:2 * P])])
    has_bq = bool(np.any(bq))
    has_bkv = bool(np.any(bk) or np.any(bv))
    has_bo = bool(np.any(bo))

    nc = _build(nblk, has_bq, has_bkv, has_bo)

    in_maps = []
    for r in range(NCORES):
        in_maps.append({
            "recp": recp[r],
            "keysT": keysT[r],
            "wk8": wk8,
            "wvhi": wvhi,
            "wvlo": wvlo,
            "wq": wq_b,
            "wo": wo_b,
            "bq": to_bf(bq[None, :]),
            "bkv": to_bf(np.concatenate([bk * KSCALE, bv * KSCALE])[None, :]),
            "bo": to_bf(bo[None, :]),
        })

    global _last_in_maps
    _last_in_maps = in_maps
    res = run_bass_kernel_spmd(nc, in_maps, core_ids=list(range(NCORES)))

    out_full = np.zeros((C, OUT_D), np.float32)
    for r in range(NCORES):
        o = res.results[r]["out"]
        for b, (base, nseg) in enumerate(cores[r]):
            if nseg > 0:
                out_full[base:base + nseg] = o[b * P:b * P + nseg]
    return out_full

